# revision 19
# baseline (speedup 1.0000x reference)
"""Trainium2 Bass kernel for MemorySpatialAttention.

Math (per batch b):
  f = LeakyReLU_0.1(BN(conv(x)))  with conv = full-length dot -> x[N,L] @ W[L,H]
  sim = f_in @ f_mem^T  banded to |i-j| <= 8 (17 neighbors, clamped at edges)
  attn = softmax_band(sim);  out = 0.5*x + 0.5*(attn @ mem)

Sharding: data-parallel over batch B=8 -> one batch per NeuronCore, no
collectives.

Per-core structure: features in [H=128, N] layout (L/H contractions on
partitions). Queries are tiled 112 rows at a time with a uniform 128-wide
key window [112t-8, 112t+120) so each tile is exactly one matmul, one PE
transpose and one attn@mem matmul (no K-splits). fmT/fiT are zero-padded so
windows never leave the tensor; band masking is an additive -1e10 constant.
Tiles are processed in groups of 4 sharing one PSUM bank, so the softmax
(mask-add, row-max, subtract, exp, row-sum, reciprocal, normalize, blend)
runs as one batched instruction per group, with per-tile scalars applied
through zero-stride broadcast reads. mem rows are pre-staged window-aligned
(memNB) from a partition-major DRAM image, so no per-tile DMA exists at all.
"""
import sys
sys.path.insert(0, '/opt/trn_rl_repo')

import numpy as np

B, N, C, L, H = 8, 2048, 1, 56, 128
NB, HALF = 17, 8
RT = 112              # query rows per tile
WIN = 128             # key window per tile
T = (N + RT - 1) // RT  # 19 tiles (last partial: 32 rows)
GRP = 8
NG = (T + GRP - 1) // GRP  # 3 groups (8, 8, 3 tiles)
NPAD = RT * T         # 2128
RATE = 0.5
BN_EPS = 1e-5
NEG_SLOPE = 0.1
FI_PAD = NPAD         # fiT cols (2128)
FM_PAD = HALF + N + (RT * (T - 1) + WIN - N)  # 8 + 2048 + 88 = 2144

_cache = {}


def _build_program():
    import concourse.bass as bass
    import concourse.bacc as bacc
    import concourse.tile as tile
    from concourse import mybir
    from concourse.masks import make_identity

    F32 = mybir.dt.float32
    F32R = mybir.dt.float32r
    BF16 = mybir.dt.bfloat16
    AF = mybir.ActivationFunctionType
    AX = mybir.AxisListType

    def bcast(ap_slice, n):
        return bass.AP(tensor=ap_slice.tensor, offset=ap_slice.offset,
                       ap=[*ap_slice.ap, [0, n]])

    nc = bacc.Bacc("TRN2", target_bir_lowering=False, debug=False)

    xT = nc.dram_tensor("xT", [L, N], F32R, kind="ExternalInput")
    memT = nc.dram_tensor("memT", [L, N], F32R, kind="ExternalInput")
    wT = nc.dram_tensor("wT", [L, H], F32R, kind="ExternalInput")
    scb = nc.dram_tensor("scb", [H, 2], F32, kind="ExternalInput")
    maskG = nc.dram_tensor("maskG", [RT, 3 * GRP * WIN], BF16, kind="ExternalInput")
    memNB = nc.dram_tensor("memNB", [128, T * L], BF16, kind="ExternalInput")
    xhp = nc.dram_tensor("xhp", [RT, T * L], F32, kind="ExternalInput")
    out = nc.dram_tensor("out", [RT, T * L], F32, kind="ExternalOutput")

    with tile.TileContext(nc) as tc:
        with tc.tile_pool(name="consts", bufs=1) as consts, \
             tc.tile_pool(name="work", bufs=3) as work, \
             tc.tile_pool(name="pbig", bufs=2, space="PSUM") as pbig, \
             tc.tile_pool(name="pt", bufs=2, space="PSUM") as pt_pool, \
             tc.tile_pool(name="pc", bufs=2, space="PSUM") as pc_pool:

            xT_s = consts.tile([L, N], F32R)
            memT_s = consts.tile([L, N], F32R)
            wT_s = consts.tile([L, H], F32R)
            scb_s = consts.tile([H, 2], F32)
            maskG_s = consts.tile([RT, 3, GRP, WIN], BF16)
            memNB_s = consts.tile([128, T, L], BF16)
            xh_s = consts.tile([RT, T, L], F32)
            ident = consts.tile([RT, RT], BF16)
            fiT = consts.tile([H, FI_PAD], F32)
            fmT = consts.tile([H, FM_PAD], F32)
            simS = consts.tile([RT, T, WIN], F32)
            simB = consts.tile([RT, T, WIN], BF16)
            EB = consts.tile([RT, T, WIN], BF16)
            negmax = consts.tile([RT, T], F32)
            rinv = consts.tile([RT, T], F32)
            outn = consts.tile([RT, T, L], F32)

            # gating DMAs first (features need wT/xT/memT); bulk consts on
            # other queues so one DGE ring doesn't serialize the transfers
            nc.sync.dma_start(out=wT_s, in_=wT.ap())
            for c in range(4):
                cs = slice(c * 512, (c + 1) * 512)
                nc.sync.dma_start(out=xT_s[:, cs], in_=xT.ap()[:, cs])
                nc.sync.dma_start(out=memT_s[:, cs], in_=memT.ap()[:, cs])
            nc.sync.dma_start(out=scb_s, in_=scb.ap())
            nc.gpsimd.dma_start(out=maskG_s, in_=maskG.ap().rearrange(
                "p (t g w) -> p t g w", g=GRP, w=WIN))
            nc.scalar.dma_start(out=memNB_s, in_=memNB.ap().rearrange(
                "p (t d) -> p t d", d=L))
            nc.gpsimd.dma_start(out=xh_s, in_=xhp.ap().rearrange(
                "p (t d) -> p t d", d=L))
            make_identity(nc, ident)
            nc.vector.memset(fmT[:, 0:HALF], 0.0)
            nc.vector.memset(fmT[:, HALF + N:FM_PAD], 0.0)
            nc.vector.memset(fiT[:, N:FI_PAD], 0.0)

            # ---- features: f^T = PRelu(BN(W^T @ x^T)), fp32r matmuls ----
            CH = 512
            for c in range(2):
                for src, dst, off in ((xT_s, fiT, 0), (memT_s, fmT, HALF)):
                    psF = pbig.tile([128, 2 * CH], F32, tag="pbig", name="psF")
                    for h in range(2):
                        nc.tensor.matmul(
                            psF[:, h * CH:(h + 1) * CH], lhsT=wT_s,
                            rhs=src[:, (2 * c + h) * CH:(2 * c + h + 1) * CH],
                            start=True, stop=True)
                    nc.scalar.activation(dst[:, off + c * 2 * CH:off + (c + 1) * 2 * CH],
                                         psF, AF.Prelu, bias=scb_s[:, 1:2],
                                         scale=scb_s[:, 0:1], alpha=NEG_SLOPE)

            # ---- banded attention in groups of GRP tiles ----
            for g in range(NG):
                tiles = list(range(g * GRP, min((g + 1) * GRP, T)))
                K = len(tiles)
                gt = 0 if g == 0 else (2 if g == NG - 1 else 1)
                t0 = tiles[0]

                psA = pbig.tile([RT, GRP, WIN], F32, tag="pbig", name="psA")
                for k, t in enumerate(tiles):
                    nc.tensor.matmul(psA[:, k, :], lhsT=fiT[:, RT * t:RT * (t + 1)],
                                     rhs=fmT[:, RT * t:RT * t + WIN],
                                     start=True, stop=True)

                sS = simS[:, t0:t0 + K, :]
                nc.vector.tensor_add(sS, psA[:, 0:K, :], maskG_s[:, gt, 0:K, :])
                nc.vector.reduce_max(negmax[:, t0:t0 + K], sS, axis=AX.X, negate=True)
                nc.vector.tensor_add(simB[:, t0:t0 + K, :], sS,
                                     bcast(negmax[:, t0:t0 + K], WIN))
                nc.scalar.activation(EB[:, t0:t0 + K, :], simB[:, t0:t0 + K, :], AF.Exp)
                nc.vector.reduce_sum(rinv[:, t0:t0 + K], EB[:, t0:t0 + K, :], axis=AX.X)
                nc.vector.reciprocal(rinv[:, t0:t0 + K], rinv[:, t0:t0 + K])

                psT = pt_pool.tile([128, GRP, RT], BF16)
                for k, t in enumerate(tiles):
                    nc.tensor.transpose(psT[:, k, :], EB[:, t, :], ident)
                attnT = work.tile([128, GRP, RT], BF16)
                if g % 2 == 0:
                    nc.scalar.copy(attnT[:, 0:K, :], psT[:, 0:K, :])
                else:
                    nc.vector.tensor_copy(attnT[:, 0:K, :], psT[:, 0:K, :])

                psC = pc_pool.tile([RT, GRP, L], F32)
                for k, t in enumerate(tiles):
                    nc.tensor.matmul(psC[:, k, :], lhsT=attnT[:, k, :],
                                     rhs=memNB_s[:, t, :], start=True, stop=True)

                tmp = work.tile([RT, GRP, L], F32)
                nc.vector.tensor_mul(tmp[:, 0:K, :], psC[:, 0:K, :],
                                     bcast(rinv[:, t0:t0 + K], L))
                nc.vector.tensor_add(outn[:, t0:t0 + K, :], tmp[:, 0:K, :],
                                     xh_s[:, t0:t0 + K, :])

            for h in range(2):
                lo = h * 10
                hi = min(T, lo + 10)
                nc.sync.dma_start(
                    out=out.ap().rearrange("p (t d) -> p t d", d=L)[:, lo:hi, :],
                    in_=outn[:, lo:hi, :])

    nc.compile()
    return nc


def _host_prep(input, state_memory, conv_w, conv_b, bn_gamma, bn_beta, bn_mean, bn_var):
    from ml_dtypes import bfloat16

    s = (bn_gamma / np.sqrt(bn_var + BN_EPS)).astype(np.float32)
    bias_h = ((conv_b - bn_mean) * s + bn_beta).astype(np.float32)
    scb = np.ascontiguousarray(np.stack([s, bias_h], axis=1))          # [H, 2]
    wT = np.ascontiguousarray(conv_w[:, 0, :].T).astype(np.float32)    # [L, H]

    # Per-tile mask [RT, WIN]: tile t covers queries i = RT*t + r, keys
    # j = RT*t - 8 + c  (c = local col). Band |i-j| <= 8 -> c in [r, r+16],
    # clipped by 0 <= j < N and i < N.
    def tile_mask(t):
        m = np.full((RT, WIN), -1e10, dtype=np.float32)
        for r in range(RT):
            i = RT * t + r
            if i >= N:
                continue
            lo = max(i - HALF, 0) - (RT * t - HALF)
            hi = min(i + HALF, N - 1) - (RT * t - HALF)
            m[r, lo:hi + 1] = 0.0
        return m

    mids = tile_mask(1)
    maskG = np.empty((3, GRP, RT, WIN), dtype=np.float32)
    maskG[:] = mids[None, None]
    maskG[0, 0] = tile_mask(0)
    maskG[2, T - 1 - (NG - 1) * GRP] = tile_mask(T - 1)
    maskG = np.ascontiguousarray(maskG.transpose(2, 0, 1, 3).reshape(RT, -1))

    in_maps = []
    for b in range(B):
        x = np.ascontiguousarray(input[b, :, 0, :]).astype(np.float32)
        mem = np.ascontiguousarray(state_memory[b, :, 0, :]).astype(np.float32)
        # window-aligned mem blocks: block t = rows [RT*t-8, RT*t+120)
        mnb = np.zeros((T, 128, L), dtype=np.float32)
        half_mem = (1.0 - RATE) * mem
        for t in range(T):
            lo = RT * t - HALF
            a, bnd = max(0, lo), min(N, lo + 128)
            if a < bnd:
                mnb[t, a - lo:bnd - lo] = half_mem[a:bnd]
        xh = np.zeros((T, RT, L), dtype=np.float32)
        xh.reshape(-1, L)[:N] = RATE * x
        in_maps.append({
            "xT": np.ascontiguousarray(x.T),
            "memT": np.ascontiguousarray(mem.T),
            "wT": wT,
            "scb": scb,
            "maskG": maskG.astype(bfloat16),
            "memNB": np.ascontiguousarray(
                mnb.transpose(1, 0, 2).reshape(128, -1)).astype(bfloat16),
            "xhp": np.ascontiguousarray(xh.transpose(1, 0, 2).reshape(RT, -1)),
        })
    return in_maps


def run(inputs, trace=False):
    from concourse.bass_utils import run_bass_kernel_spmd
    if "nc" not in _cache:
        _cache["nc"] = _build_program()
    nc = _cache["nc"]
    in_maps = _host_prep(**inputs)
    res = run_bass_kernel_spmd(nc, in_maps, core_ids=list(range(B)), trace=trace)
    out = np.empty((B, N, C, L), dtype=np.float32)
    for b in range(B):
        o = res.results[b]["out"].reshape(RT, T, L).transpose(1, 0, 2)
        out[b] = o.reshape(NPAD, L)[:N].reshape(N, C, L)
    return out, res


def kernel(**inputs):
    out, _ = run(inputs, trace=False)
    return out


# revision 20
# speedup vs baseline: 1.0033x; 1.0033x over previous
"""Trainium2 Bass kernel for MemorySpatialAttention.

Math (per batch b):
  f = LeakyReLU_0.1(BN(conv(x)))  with conv = full-length dot -> x[N,L] @ W[L,H]
  sim = f_in @ f_mem^T  banded to |i-j| <= 8 (17 neighbors, clamped at edges)
  attn = softmax_band(sim);  out = 0.5*x + 0.5*(attn @ mem)

Sharding: data-parallel over batch B=8 -> one batch per NeuronCore, no
collectives.

Per-core structure: features in [H=128, N] layout (L/H contractions on
partitions). Queries are tiled 112 rows at a time with a uniform 128-wide
key window [112t-8, 112t+120) so each tile is exactly one matmul, one PE
transpose and one attn@mem matmul (no K-splits). fmT/fiT are zero-padded so
windows never leave the tensor; band masking is an additive -1e10 constant.
Tiles are processed in groups of 4 sharing one PSUM bank, so the softmax
(mask-add, row-max, subtract, exp, row-sum, reciprocal, normalize, blend)
runs as one batched instruction per group, with per-tile scalars applied
through zero-stride broadcast reads. mem rows are pre-staged window-aligned
(memNB) from a partition-major DRAM image, so no per-tile DMA exists at all.
"""
import sys
sys.path.insert(0, '/opt/trn_rl_repo')

import numpy as np

B, N, C, L, H = 8, 2048, 1, 56, 128
NB, HALF = 17, 8
RT = 112              # query rows per tile
WIN = 128             # key window per tile
T = (N + RT - 1) // RT  # 19 tiles (last partial: 32 rows)
GRP = 8
NG = (T + GRP - 1) // GRP  # 3 groups (8, 8, 3 tiles)
NPAD = RT * T         # 2128
RATE = 0.5
BN_EPS = 1e-5
NEG_SLOPE = 0.1
FI_PAD = NPAD         # fiT cols (2128)
FM_PAD = HALF + N + (RT * (T - 1) + WIN - N)  # 8 + 2048 + 88 = 2144

_cache = {}


def _build_program():
    import concourse.bass as bass
    import concourse.bacc as bacc
    import concourse.tile as tile
    from concourse import mybir
    from concourse.masks import make_identity

    F32 = mybir.dt.float32
    F32R = mybir.dt.float32r
    BF16 = mybir.dt.bfloat16
    AF = mybir.ActivationFunctionType
    AX = mybir.AxisListType

    def bcast(ap_slice, n):
        return bass.AP(tensor=ap_slice.tensor, offset=ap_slice.offset,
                       ap=[*ap_slice.ap, [0, n]])

    nc = bacc.Bacc("TRN2", target_bir_lowering=False, debug=False)

    xm = nc.dram_tensor("xm", [L, 2 * N], F32R, kind="ExternalInput")
    wT = nc.dram_tensor("wT", [L, H], F32R, kind="ExternalInput")
    scb = nc.dram_tensor("scb", [H, 2], F32, kind="ExternalInput")
    maskG = nc.dram_tensor("maskG", [RT, 3 * GRP * WIN], BF16, kind="ExternalInput")
    memNB = nc.dram_tensor("memNB", [128, T * L], BF16, kind="ExternalInput")
    xhp = nc.dram_tensor("xhp", [RT, T * L], F32, kind="ExternalInput")
    out = nc.dram_tensor("out", [RT, T * L], F32, kind="ExternalOutput")

    with tile.TileContext(nc) as tc:
        with tc.tile_pool(name="consts", bufs=1) as consts, \
             tc.tile_pool(name="work", bufs=3) as work, \
             tc.tile_pool(name="pbig", bufs=3, space="PSUM") as pbig, \
             tc.tile_pool(name="ptc", bufs=2, space="PSUM") as ptc:

            xm_s = consts.tile([L, 2 * N], F32R)
            xT_s = xm_s[:, 0:N]
            memT_s = xm_s[:, N:2 * N]
            wT_s = consts.tile([L, H], F32R)
            scb_s = consts.tile([H, 2], F32)
            maskG_s = consts.tile([RT, 3, GRP, WIN], BF16)
            memNB_s = consts.tile([128, T, L], BF16)
            xh_s = consts.tile([RT, T, L], F32)
            ident = consts.tile([RT, RT], BF16)
            fiT = consts.tile([H, FI_PAD], F32)
            fmT = consts.tile([H, FM_PAD], F32)
            simS = consts.tile([RT, T, WIN], F32)
            simB = consts.tile([RT, T, WIN], BF16)
            EB = consts.tile([RT, T, WIN], BF16)
            negmax = consts.tile([RT, T], F32)
            rinv = consts.tile([RT, T], F32)
            outn = consts.tile([RT, T, L], F32)

            # gating DMAs first (features need wT/xT/memT); bulk consts on
            # other queues so one DGE ring doesn't serialize the transfers
            nc.sync.dma_start(out=wT_s, in_=wT.ap())
            nc.sync.dma_start(out=xm_s, in_=xm.ap())
            nc.sync.dma_start(out=scb_s, in_=scb.ap())
            nc.gpsimd.dma_start(out=maskG_s, in_=maskG.ap().rearrange(
                "p (t g w) -> p t g w", g=GRP, w=WIN))
            nc.scalar.dma_start(out=memNB_s, in_=memNB.ap().rearrange(
                "p (t d) -> p t d", d=L))
            nc.gpsimd.dma_start(out=xh_s, in_=xhp.ap().rearrange(
                "p (t d) -> p t d", d=L))
            make_identity(nc, ident)
            nc.vector.memset(fmT[:, 0:HALF], 0.0)
            nc.vector.memset(fmT[:, HALF + N:FM_PAD], 0.0)
            nc.vector.memset(fiT[:, N:FI_PAD], 0.0)

            # ---- features: f^T = PRelu(BN(W^T @ x^T)), fp32r matmuls ----
            CH = 512
            for c in range(2):
                for src, dst, off in ((xT_s, fiT, 0), (memT_s, fmT, HALF)):
                    psF = pbig.tile([128, 2 * CH], F32, tag="pbig", name="psF")
                    for h in range(2):
                        nc.tensor.matmul(
                            psF[:, h * CH:(h + 1) * CH], lhsT=wT_s,
                            rhs=src[:, (2 * c + h) * CH:(2 * c + h + 1) * CH],
                            start=True, stop=True)
                    nc.scalar.activation(dst[:, off + c * 2 * CH:off + (c + 1) * 2 * CH],
                                         psF, AF.Prelu, bias=scb_s[:, 1:2],
                                         scale=scb_s[:, 0:1], alpha=NEG_SLOPE)

            # ---- banded attention in groups of GRP tiles ----
            for g in range(NG):
                tiles = list(range(g * GRP, min((g + 1) * GRP, T)))
                K = len(tiles)
                gt = 0 if g == 0 else (2 if g == NG - 1 else 1)
                t0 = tiles[0]

                psA = pbig.tile([RT, GRP, WIN], F32, tag="pbig", name="psA")
                for k, t in enumerate(tiles):
                    nc.tensor.matmul(psA[:, k, :], lhsT=fiT[:, RT * t:RT * (t + 1)],
                                     rhs=fmT[:, RT * t:RT * t + WIN],
                                     start=True, stop=True)

                sS = simS[:, t0:t0 + K, :]
                nc.vector.tensor_add(sS, psA[:, 0:K, :], maskG_s[:, gt, 0:K, :])
                nc.vector.reduce_max(negmax[:, t0:t0 + K], sS, axis=AX.X, negate=True)
                nc.vector.tensor_add(simB[:, t0:t0 + K, :], sS,
                                     bcast(negmax[:, t0:t0 + K], WIN))
                nc.scalar.activation(EB[:, t0:t0 + K, :], simB[:, t0:t0 + K, :], AF.Exp)
                nc.vector.reduce_sum(rinv[:, t0:t0 + K], EB[:, t0:t0 + K, :], axis=AX.X)
                nc.vector.reciprocal(rinv[:, t0:t0 + K], rinv[:, t0:t0 + K])

                psT = ptc.tile([128, GRP, RT], BF16, tag="ptc")
                for k, t in enumerate(tiles):
                    nc.tensor.transpose(psT[:, k, :], EB[:, t, :], ident)
                attnT = work.tile([128, GRP, RT], BF16)
                if g % 2 == 0:
                    nc.scalar.copy(attnT[:, 0:K, :], psT[:, 0:K, :])
                else:
                    nc.vector.tensor_copy(attnT[:, 0:K, :], psT[:, 0:K, :])

                psC = ptc.tile([RT, GRP, L], F32, tag="ptc")
                for k, t in enumerate(tiles):
                    nc.tensor.matmul(psC[:, k, :], lhsT=attnT[:, k, :],
                                     rhs=memNB_s[:, t, :], start=True, stop=True)

                tmp = work.tile([RT, GRP, L], F32)
                nc.vector.tensor_mul(tmp[:, 0:K, :], psC[:, 0:K, :],
                                     bcast(rinv[:, t0:t0 + K], L))
                nc.vector.tensor_add(outn[:, t0:t0 + K, :], tmp[:, 0:K, :],
                                     xh_s[:, t0:t0 + K, :])

            for h in range(2):
                lo = h * 10
                hi = min(T, lo + 10)
                nc.sync.dma_start(
                    out=out.ap().rearrange("p (t d) -> p t d", d=L)[:, lo:hi, :],
                    in_=outn[:, lo:hi, :])

    nc.compile()
    return nc


def _host_prep(input, state_memory, conv_w, conv_b, bn_gamma, bn_beta, bn_mean, bn_var):
    from ml_dtypes import bfloat16

    s = (bn_gamma / np.sqrt(bn_var + BN_EPS)).astype(np.float32)
    bias_h = ((conv_b - bn_mean) * s + bn_beta).astype(np.float32)
    scb = np.ascontiguousarray(np.stack([s, bias_h], axis=1))          # [H, 2]
    wT = np.ascontiguousarray(conv_w[:, 0, :].T).astype(np.float32)    # [L, H]

    # Per-tile mask [RT, WIN]: tile t covers queries i = RT*t + r, keys
    # j = RT*t - 8 + c  (c = local col). Band |i-j| <= 8 -> c in [r, r+16],
    # clipped by 0 <= j < N and i < N.
    def tile_mask(t):
        m = np.full((RT, WIN), -1e10, dtype=np.float32)
        for r in range(RT):
            i = RT * t + r
            if i >= N:
                continue
            lo = max(i - HALF, 0) - (RT * t - HALF)
            hi = min(i + HALF, N - 1) - (RT * t - HALF)
            m[r, lo:hi + 1] = 0.0
        return m

    mids = tile_mask(1)
    maskG = np.empty((3, GRP, RT, WIN), dtype=np.float32)
    maskG[:] = mids[None, None]
    maskG[0, 0] = tile_mask(0)
    maskG[2, T - 1 - (NG - 1) * GRP] = tile_mask(T - 1)
    maskG = np.ascontiguousarray(maskG.transpose(2, 0, 1, 3).reshape(RT, -1))

    in_maps = []
    for b in range(B):
        x = np.ascontiguousarray(input[b, :, 0, :]).astype(np.float32)
        mem = np.ascontiguousarray(state_memory[b, :, 0, :]).astype(np.float32)
        # window-aligned mem blocks: block t = rows [RT*t-8, RT*t+120)
        mnb = np.zeros((T, 128, L), dtype=np.float32)
        half_mem = (1.0 - RATE) * mem
        for t in range(T):
            lo = RT * t - HALF
            a, bnd = max(0, lo), min(N, lo + 128)
            if a < bnd:
                mnb[t, a - lo:bnd - lo] = half_mem[a:bnd]
        xh = np.zeros((T, RT, L), dtype=np.float32)
        xh.reshape(-1, L)[:N] = RATE * x
        in_maps.append({
            "xm": np.ascontiguousarray(np.concatenate([x.T, mem.T], axis=1)),
            "wT": wT,
            "scb": scb,
            "maskG": maskG.astype(bfloat16),
            "memNB": np.ascontiguousarray(
                mnb.transpose(1, 0, 2).reshape(128, -1)).astype(bfloat16),
            "xhp": np.ascontiguousarray(xh.transpose(1, 0, 2).reshape(RT, -1)),
        })
    return in_maps


def run(inputs, trace=False):
    from concourse.bass_utils import run_bass_kernel_spmd
    if "nc" not in _cache:
        _cache["nc"] = _build_program()
    nc = _cache["nc"]
    in_maps = _host_prep(**inputs)
    res = run_bass_kernel_spmd(nc, in_maps, core_ids=list(range(B)), trace=trace)
    out = np.empty((B, N, C, L), dtype=np.float32)
    for b in range(B):
        o = res.results[b]["out"].reshape(RT, T, L).transpose(1, 0, 2)
        out[b] = o.reshape(NPAD, L)[:N].reshape(N, C, L)
    return out, res


def kernel(**inputs):
    out, _ = run(inputs, trace=False)
    return out


# revision 23
# speedup vs baseline: 1.0293x; 1.0259x over previous
"""Trainium2 Bass kernel for MemorySpatialAttention.

Math (per batch b):
  f = LeakyReLU_0.1(BN(conv(x)))  with conv = full-length dot -> x[N,L] @ W[L,H]
  sim = f_in @ f_mem^T  banded to |i-j| <= 8 (17 neighbors, clamped at edges)
  attn = softmax_band(sim);  out = 0.5*x + 0.5*(attn @ mem)

Sharding: data-parallel over batch B=8 -> one batch per NeuronCore, no
collectives.

Per-core structure: features in [H=128, N] layout (L/H contractions on
partitions). Queries are tiled 112 rows at a time with a uniform 128-wide
key window [112t-8, 112t+120) so each tile is exactly one matmul, one PE
transpose and one attn@mem matmul (no K-splits). fmT/fiT are zero-padded so
windows never leave the tensor; band masking is an additive -1e10 constant.
Tiles are processed in groups of 4 sharing one PSUM bank, so the softmax
(mask-add, row-max, subtract, exp, row-sum, reciprocal, normalize, blend)
runs as one batched instruction per group, with per-tile scalars applied
through zero-stride broadcast reads. mem rows are pre-staged window-aligned
(memNB) from a partition-major DRAM image, so no per-tile DMA exists at all.
"""
import sys
sys.path.insert(0, '/opt/trn_rl_repo')

import numpy as np

B, N, C, L, H = 8, 2048, 1, 56, 128
NB, HALF = 17, 8
RT = 112              # query rows per tile
WIN = 128             # key window per tile
T = (N + RT - 1) // RT  # 19 tiles (last partial: 32 rows)
GRP = 8
NG = (T + GRP - 1) // GRP  # 3 groups (8, 8, 3 tiles)
NPAD = RT * T         # 2128
RATE = 0.5
BN_EPS = 1e-5
NEG_SLOPE = 0.1
FI_PAD = NPAD         # fiT cols (2128)
FM_PAD = HALF + N + (RT * (T - 1) + WIN - N)  # 8 + 2048 + 88 = 2144

_cache = {}


def _build_program():
    import concourse.bass as bass
    import concourse.bacc as bacc
    import concourse.tile as tile
    from concourse import mybir
    
    F32 = mybir.dt.float32
    F32R = mybir.dt.float32r
    BF16 = mybir.dt.bfloat16
    AF = mybir.ActivationFunctionType
    AX = mybir.AxisListType

    def bcast(ap_slice, n):
        return bass.AP(tensor=ap_slice.tensor, offset=ap_slice.offset,
                       ap=[*ap_slice.ap, [0, n]])

    nc = bacc.Bacc("TRN2", target_bir_lowering=False, debug=False)

    xm = nc.dram_tensor("xm", [L, 2 * N], F32R, kind="ExternalInput")
    idt = nc.dram_tensor("idt", [RT, RT], BF16, kind="ExternalInput")
    wT = nc.dram_tensor("wT", [L, H], F32R, kind="ExternalInput")
    scb = nc.dram_tensor("scb", [H, 2], F32, kind="ExternalInput")
    maskG = nc.dram_tensor("maskG", [RT, 3 * GRP * WIN], BF16, kind="ExternalInput")
    memNB = nc.dram_tensor("memNB", [128, T * (L + 1)], BF16, kind="ExternalInput")
    xhp = nc.dram_tensor("xhp", [RT, T * L], F32, kind="ExternalInput")
    out = nc.dram_tensor("out", [RT, T * L], F32, kind="ExternalOutput")

    with tile.TileContext(nc) as tc:
        with tc.tile_pool(name="consts", bufs=1) as consts, \
             tc.tile_pool(name="work", bufs=3) as work, \
             tc.tile_pool(name="pbig", bufs=3, space="PSUM") as pbig, \
             tc.tile_pool(name="ptc", bufs=2, space="PSUM") as ptc:

            xm_s = consts.tile([L, 2 * N], F32R)
            wT_s = consts.tile([L, H], F32R)
            scb_s = consts.tile([H, 2], F32)
            maskG_s = consts.tile([RT, 3, GRP, WIN], BF16)
            memNB_s = consts.tile([128, T, L + 1], BF16)
            xh_s = consts.tile([RT, T, L], F32)
            ident = consts.tile([RT, RT], BF16)
            fiT = consts.tile([H, FI_PAD], F32)
            fmT = consts.tile([H, FM_PAD], F32)
            simS = consts.tile([RT, T, WIN], F32)
            simB = consts.tile([RT, T, WIN], BF16)
            EB = consts.tile([RT, T, WIN], BF16)
            negmax = consts.tile([RT, T], F32)
            rinv = consts.tile([RT, T], F32)
            outn = consts.tile([RT, T, L], F32)

            # gating DMAs first (features need wT/xT/memT); bulk consts on
            # other queues so one DGE ring doesn't serialize the transfers
            nc.sync.dma_start(out=wT_s, in_=wT.ap())
            nc.sync.dma_start(out=xm_s[:, 0:N], in_=xm.ap()[:, 0:N])
            nc.sync.dma_start(out=xm_s[:, N:2 * N], in_=xm.ap()[:, N:2 * N])
            nc.sync.dma_start(out=scb_s, in_=scb.ap())
            nc.gpsimd.dma_start(out=maskG_s, in_=maskG.ap().rearrange(
                "p (t g w) -> p t g w", g=GRP, w=WIN))
            nc.scalar.dma_start(out=memNB_s, in_=memNB.ap().rearrange(
                "p (t d) -> p t d", d=L + 1))
            nc.scalar.dma_start(out=ident, in_=idt.ap())
            nc.gpsimd.dma_start(out=xh_s, in_=xhp.ap().rearrange(
                "p (t d) -> p t d", d=L))
            nc.vector.memset(fmT[:, 0:HALF], 0.0)
            nc.vector.memset(fmT[:, HALF + N:FM_PAD], 0.0)
            nc.vector.memset(fiT[:, N:FI_PAD], 0.0)

            # ---- features: f^T = PRelu(BN(W^T @ x^T)), fp32r matmuls ----
            CH = 512
            for c in range(2):
                for m, (dst, off) in enumerate(((fiT, 0), (fmT, HALF))):
                    psF = pbig.tile([128, 2 * CH], F32, tag="pbig", name="psF")
                    for h in range(2):
                        ox = 2048 * c + 1024 * m + CH * h
                        nc.tensor.matmul(
                            psF[:, h * CH:(h + 1) * CH], lhsT=wT_s,
                            rhs=xm_s[:, ox:ox + CH], start=True, stop=True)
                    nc.scalar.activation(dst[:, off + c * 2 * CH:off + (c + 1) * 2 * CH],
                                         psF, AF.Prelu, bias=scb_s[:, 1:2],
                                         scale=scb_s[:, 0:1], alpha=NEG_SLOPE)

            # ---- banded attention in groups of GRP tiles ----
            for g in range(NG):
                tiles = list(range(g * GRP, min((g + 1) * GRP, T)))
                K = len(tiles)
                gt = 0 if g == 0 else (2 if g == NG - 1 else 1)
                t0 = tiles[0]

                psA = pbig.tile([RT, GRP, WIN], F32, tag="pbig", name="psA")
                for k, t in enumerate(tiles):
                    nc.tensor.matmul(psA[:, k, :], lhsT=fiT[:, RT * t:RT * (t + 1)],
                                     rhs=fmT[:, RT * t:RT * t + WIN],
                                     start=True, stop=True)

                sS = simS[:, t0:t0 + K, :]
                nc.vector.tensor_add(sS, psA[:, 0:K, :], maskG_s[:, gt, 0:K, :])
                nc.vector.reduce_max(negmax[:, t0:t0 + K], sS, axis=AX.X, negate=True)
                nc.vector.tensor_add(simB[:, t0:t0 + K, :], sS,
                                     bcast(negmax[:, t0:t0 + K], WIN))
                nc.scalar.activation(EB[:, t0:t0 + K, :], simB[:, t0:t0 + K, :], AF.Exp)

                psT = ptc.tile([128, GRP, RT], BF16, tag="ptc")
                for k, t in enumerate(tiles):
                    nc.tensor.transpose(psT[:, k, :], EB[:, t, :], ident)
                attnT = work.tile([128, GRP, RT], BF16)
                if g % 2 == 0:
                    nc.scalar.copy(attnT[:, 0:K, :], psT[:, 0:K, :])
                else:
                    nc.vector.tensor_copy(attnT[:, 0:K, :], psT[:, 0:K, :])

                psC = ptc.tile([RT, GRP, L + 1], F32, tag="ptc")
                for k, t in enumerate(tiles):
                    nc.tensor.matmul(psC[:, k, :], lhsT=attnT[:, k, :],
                                     rhs=memNB_s[:, t, :], start=True, stop=True)

                nc.vector.reciprocal(rinv[:, t0:t0 + K], psC[:, 0:K, L])
                tmp = work.tile([RT, GRP, L], F32)
                nc.vector.tensor_mul(tmp[:, 0:K, :], psC[:, 0:K, 0:L],
                                     bcast(rinv[:, t0:t0 + K], L))
                nc.vector.tensor_add(outn[:, t0:t0 + K, :], tmp[:, 0:K, :],
                                     xh_s[:, t0:t0 + K, :])

            for h in range(2):
                lo = h * 10
                hi = min(T, lo + 10)
                nc.sync.dma_start(
                    out=out.ap().rearrange("p (t d) -> p t d", d=L)[:, lo:hi, :],
                    in_=outn[:, lo:hi, :])

    nc.compile()
    return nc


def _host_prep(input, state_memory, conv_w, conv_b, bn_gamma, bn_beta, bn_mean, bn_var):
    from ml_dtypes import bfloat16

    s = (bn_gamma / np.sqrt(bn_var + BN_EPS)).astype(np.float32)
    bias_h = ((conv_b - bn_mean) * s + bn_beta).astype(np.float32)
    scb = np.ascontiguousarray(np.stack([s, bias_h], axis=1))          # [H, 2]
    wT = np.ascontiguousarray(conv_w[:, 0, :].T).astype(np.float32)    # [L, H]

    # Per-tile mask [RT, WIN]: tile t covers queries i = RT*t + r, keys
    # j = RT*t - 8 + c  (c = local col). Band |i-j| <= 8 -> c in [r, r+16],
    # clipped by 0 <= j < N and i < N.
    def tile_mask(t):
        m = np.full((RT, WIN), -1e10, dtype=np.float32)
        for r in range(RT):
            i = RT * t + r
            if i >= N:
                continue
            lo = max(i - HALF, 0) - (RT * t - HALF)
            hi = min(i + HALF, N - 1) - (RT * t - HALF)
            m[r, lo:hi + 1] = 0.0
        return m

    mids = tile_mask(1)
    maskG = np.empty((3, GRP, RT, WIN), dtype=np.float32)
    maskG[:] = mids[None, None]
    maskG[0, 0] = tile_mask(0)
    maskG[2, T - 1 - (NG - 1) * GRP] = tile_mask(T - 1)
    maskG = np.ascontiguousarray(maskG.transpose(2, 0, 1, 3).reshape(RT, -1))

    in_maps = []
    for b in range(B):
        x = np.ascontiguousarray(input[b, :, 0, :]).astype(np.float32)
        mem = np.ascontiguousarray(state_memory[b, :, 0, :]).astype(np.float32)
        # window-aligned mem blocks: block t = rows [RT*t-8, RT*t+120)
        mnb = np.zeros((T, 128, L + 1), dtype=np.float32)
        half_mem = (1.0 - RATE) * mem
        for t in range(T):
            lo = RT * t - HALF
            a, bnd = max(0, lo), min(N, lo + 128)
            if a < bnd:
                mnb[t, a - lo:bnd - lo, 0:L] = half_mem[a:bnd]
                mnb[t, a - lo:bnd - lo, L] = 1.0
        xh = np.zeros((T, RT, L), dtype=np.float32)
        xh.reshape(-1, L)[:N] = RATE * x
        in_maps.append({
            "xm": np.ascontiguousarray(np.concatenate(
                [x.T[:, 0:1024], mem.T[:, 0:1024],
                 x.T[:, 1024:2048], mem.T[:, 1024:2048]], axis=1)),
            "idt": np.eye(RT, dtype=np.float32).astype(bfloat16),
            "wT": wT,
            "scb": scb,
            "maskG": maskG.astype(bfloat16),
            "memNB": np.ascontiguousarray(
                mnb.transpose(1, 0, 2).reshape(128, -1)).astype(bfloat16),
            "xhp": np.ascontiguousarray(xh.transpose(1, 0, 2).reshape(RT, -1)),
        })
    return in_maps


def run(inputs, trace=False):
    from concourse.bass_utils import run_bass_kernel_spmd
    if "nc" not in _cache:
        _cache["nc"] = _build_program()
    nc = _cache["nc"]
    in_maps = _host_prep(**inputs)
    res = run_bass_kernel_spmd(nc, in_maps, core_ids=list(range(B)), trace=trace)
    out = np.empty((B, N, C, L), dtype=np.float32)
    for b in range(B):
        o = res.results[b]["out"].reshape(RT, T, L).transpose(1, 0, 2)
        out[b] = o.reshape(NPAD, L)[:N].reshape(N, C, L)
    return out, res


def kernel(**inputs):
    out, _ = run(inputs, trace=False)
    return out


# revision 24
# speedup vs baseline: 1.0587x; 1.0286x over previous
"""Trainium2 Bass kernel for MemorySpatialAttention.

Math (per batch b):
  f = LeakyReLU_0.1(BN(conv(x)))  with conv = full-length dot -> x[N,L] @ W[L,H]
  sim = f_in @ f_mem^T  banded to |i-j| <= 8 (17 neighbors, clamped at edges)
  attn = softmax_band(sim);  out = 0.5*x + 0.5*(attn @ mem)

Sharding: data-parallel over batch B=8 -> one batch per NeuronCore, no
collectives.

Per-core structure: features in [H=128, N] layout (L/H contractions on
partitions). Queries are tiled 112 rows at a time with a uniform 128-wide
key window [112t-8, 112t+120) so each tile is exactly one matmul, one PE
transpose and one attn@mem matmul (no K-splits). fmT/fiT are zero-padded so
windows never leave the tensor; band masking is an additive -1e10 constant.
Tiles are processed in groups of 4 sharing one PSUM bank, so the softmax
(mask-add, row-max, subtract, exp, row-sum, reciprocal, normalize, blend)
runs as one batched instruction per group, with per-tile scalars applied
through zero-stride broadcast reads. mem rows are pre-staged window-aligned
(memNB) from a partition-major DRAM image, so no per-tile DMA exists at all.
"""
import sys
sys.path.insert(0, '/opt/trn_rl_repo')

import numpy as np

B, N, C, L, H = 8, 2048, 1, 56, 128
NB, HALF = 17, 8
RT = 112              # query rows per tile
WIN = 128             # key window per tile
T = (N + RT - 1) // RT  # 19 tiles (last partial: 32 rows)
GRP = 8
NG = (T + GRP - 1) // GRP  # 3 groups (8, 8, 3 tiles)
NPAD = RT * T         # 2128
RATE = 0.5
BN_EPS = 1e-5
NEG_SLOPE = 0.1
FI_PAD = NPAD         # fiT cols (2128)
FM_PAD = HALF + N + (RT * (T - 1) + WIN - N)  # 8 + 2048 + 88 = 2144

_cache = {}


def _build_program():
    import concourse.bass as bass
    import concourse.bacc as bacc
    import concourse.tile as tile
    from concourse import mybir
    
    F32 = mybir.dt.float32
    F32R = mybir.dt.float32r
    BF16 = mybir.dt.bfloat16
    AF = mybir.ActivationFunctionType
    AX = mybir.AxisListType

    def bcast(ap_slice, n):
        return bass.AP(tensor=ap_slice.tensor, offset=ap_slice.offset,
                       ap=[*ap_slice.ap, [0, n]])

    nc = bacc.Bacc("TRN2", target_bir_lowering=False, debug=False)

    xmA = nc.dram_tensor("xmA", [L, H + N], F32R, kind="ExternalInput")
    xmB = nc.dram_tensor("xmB", [L, N], F32R, kind="ExternalInput")
    idt = nc.dram_tensor("idt", [RT, RT], BF16, kind="ExternalInput")
    scb = nc.dram_tensor("scb", [H, 2], F32, kind="ExternalInput")
    maskG = nc.dram_tensor("maskG", [RT, 3 * GRP * WIN], BF16, kind="ExternalInput")
    memNB = nc.dram_tensor("memNB", [128, T * (L + 1)], BF16, kind="ExternalInput")
    xhp = nc.dram_tensor("xhp", [RT, T * L], F32, kind="ExternalInput")
    out = nc.dram_tensor("out", [RT, T * L], F32, kind="ExternalOutput")

    with tile.TileContext(nc) as tc:
        with tc.tile_pool(name="consts", bufs=1) as consts, \
             tc.tile_pool(name="work", bufs=3) as work, \
             tc.tile_pool(name="pbig", bufs=3, space="PSUM") as pbig, \
             tc.tile_pool(name="ptc", bufs=2, space="PSUM") as ptc:

            xmA_s = consts.tile([L, H + N], F32R)
            xmB_s = consts.tile([L, N], F32R)
            wT_s = xmA_s[:, 0:H]
            scb_s = consts.tile([H, 2], F32)
            maskG_s = consts.tile([RT, 3, GRP, WIN], BF16)
            memNB_s = consts.tile([128, T, L + 1], BF16)
            xh_s = consts.tile([RT, T, L], F32)
            ident = consts.tile([RT, RT], BF16)
            fiT = consts.tile([H, FI_PAD], F32)
            fmT = consts.tile([H, FM_PAD], F32)
            simS = consts.tile([RT, T, WIN], F32)
            simB = consts.tile([RT, T, WIN], BF16)
            EB = consts.tile([RT, T, WIN], BF16)
            negmax = consts.tile([RT, T], F32)
            rinv = consts.tile([RT, T], F32)
            outn = consts.tile([RT, T, L], F32)

            # critical chain: one packed DMA [wT | xT-c0 | memT-c0] gates the
            # first feature matmuls; everything else on other queues
            nc.sync.dma_start(out=xmA_s, in_=xmA.ap())
            nc.sync.dma_start(out=maskG_s, in_=maskG.ap().rearrange(
                "p (t g w) -> p t g w", g=GRP, w=WIN))
            nc.scalar.dma_start(out=scb_s, in_=scb.ap())
            nc.scalar.dma_start(out=ident, in_=idt.ap())
            nc.scalar.dma_start(out=memNB_s, in_=memNB.ap().rearrange(
                "p (t d) -> p t d", d=L + 1))
            nc.scalar.dma_start(out=xmB_s, in_=xmB.ap())
            nc.gpsimd.dma_start(out=xh_s, in_=xhp.ap().rearrange(
                "p (t d) -> p t d", d=L))
            nc.vector.memset(fmT[:, 0:HALF], 0.0)
            nc.vector.memset(fmT[:, HALF + N:FM_PAD], 0.0)
            nc.vector.memset(fiT[:, N:FI_PAD], 0.0)

            # ---- features: f^T = PRelu(BN(W^T @ x^T)), fp32r matmuls ----
            CH = 512
            for c in range(2):
                for m, (dst, off) in enumerate(((fiT, 0), (fmT, HALF))):
                    psF = pbig.tile([128, 2 * CH], F32, tag="pbig", name="psF")
                    for h in range(2):
                        srcp = xmA_s[:, H + 1024 * m + CH * h:H + 1024 * m + CH * (h + 1)] \
                            if c == 0 else \
                            xmB_s[:, 1024 * m + CH * h:1024 * m + CH * (h + 1)]
                        nc.tensor.matmul(
                            psF[:, h * CH:(h + 1) * CH], lhsT=wT_s,
                            rhs=srcp, start=True, stop=True)
                    nc.scalar.activation(dst[:, off + c * 2 * CH:off + (c + 1) * 2 * CH],
                                         psF, AF.Prelu, bias=scb_s[:, 1:2],
                                         scale=scb_s[:, 0:1], alpha=NEG_SLOPE)

            # ---- banded attention in groups of GRP tiles ----
            for g in range(NG):
                tiles = list(range(g * GRP, min((g + 1) * GRP, T)))
                K = len(tiles)
                gt = 0 if g == 0 else (2 if g == NG - 1 else 1)
                t0 = tiles[0]

                psA = pbig.tile([RT, GRP, WIN], F32, tag="pbig", name="psA")
                for k, t in enumerate(tiles):
                    nc.tensor.matmul(psA[:, k, :], lhsT=fiT[:, RT * t:RT * (t + 1)],
                                     rhs=fmT[:, RT * t:RT * t + WIN],
                                     start=True, stop=True)

                sS = simS[:, t0:t0 + K, :]
                nc.vector.tensor_add(sS, psA[:, 0:K, :], maskG_s[:, gt, 0:K, :])
                nc.vector.reduce_max(negmax[:, t0:t0 + K], sS, axis=AX.X, negate=True)
                nc.vector.tensor_add(simB[:, t0:t0 + K, :], sS,
                                     bcast(negmax[:, t0:t0 + K], WIN))
                nc.scalar.activation(EB[:, t0:t0 + K, :], simB[:, t0:t0 + K, :], AF.Exp)

                psT = ptc.tile([128, GRP, RT], BF16, tag="ptc")
                for k, t in enumerate(tiles):
                    nc.tensor.transpose(psT[:, k, :], EB[:, t, :], ident)
                attnT = work.tile([128, GRP, RT], BF16)
                nc.scalar.copy(attnT[:, 0:K, :], psT[:, 0:K, :])

                psC = ptc.tile([RT, GRP, L + 1], F32, tag="ptc")
                for k, t in enumerate(tiles):
                    nc.tensor.matmul(psC[:, k, :], lhsT=attnT[:, k, :],
                                     rhs=memNB_s[:, t, :], start=True, stop=True)

                nc.vector.reciprocal(rinv[:, t0:t0 + K], psC[:, 0:K, L])
                tmp = work.tile([RT, GRP, L], F32)
                nc.vector.tensor_mul(tmp[:, 0:K, :], psC[:, 0:K, 0:L],
                                     bcast(rinv[:, t0:t0 + K], L))
                nc.vector.tensor_add(outn[:, t0:t0 + K, :], tmp[:, 0:K, :],
                                     xh_s[:, t0:t0 + K, :])

            for h in range(2):
                lo = h * 10
                hi = min(T, lo + 10)
                nc.sync.dma_start(
                    out=out.ap().rearrange("p (t d) -> p t d", d=L)[:, lo:hi, :],
                    in_=outn[:, lo:hi, :])

    nc.compile()
    return nc


def _host_prep(input, state_memory, conv_w, conv_b, bn_gamma, bn_beta, bn_mean, bn_var):
    from ml_dtypes import bfloat16

    s = (bn_gamma / np.sqrt(bn_var + BN_EPS)).astype(np.float32)
    bias_h = ((conv_b - bn_mean) * s + bn_beta).astype(np.float32)
    scb = np.ascontiguousarray(np.stack([s, bias_h], axis=1))          # [H, 2]
    wT = np.ascontiguousarray(conv_w[:, 0, :].T).astype(np.float32)    # [L, H]

    # Per-tile mask [RT, WIN]: tile t covers queries i = RT*t + r, keys
    # j = RT*t - 8 + c  (c = local col). Band |i-j| <= 8 -> c in [r, r+16],
    # clipped by 0 <= j < N and i < N.
    def tile_mask(t):
        m = np.full((RT, WIN), -1e10, dtype=np.float32)
        for r in range(RT):
            i = RT * t + r
            if i >= N:
                continue
            lo = max(i - HALF, 0) - (RT * t - HALF)
            hi = min(i + HALF, N - 1) - (RT * t - HALF)
            m[r, lo:hi + 1] = 0.0
        return m

    mids = tile_mask(1)
    maskG = np.empty((3, GRP, RT, WIN), dtype=np.float32)
    maskG[:] = mids[None, None]
    maskG[0, 0] = tile_mask(0)
    maskG[2, T - 1 - (NG - 1) * GRP] = tile_mask(T - 1)
    maskG = np.ascontiguousarray(maskG.transpose(2, 0, 1, 3).reshape(RT, -1))

    in_maps = []
    for b in range(B):
        x = np.ascontiguousarray(input[b, :, 0, :]).astype(np.float32)
        mem = np.ascontiguousarray(state_memory[b, :, 0, :]).astype(np.float32)
        # window-aligned mem blocks: block t = rows [RT*t-8, RT*t+120)
        mnb = np.zeros((T, 128, L + 1), dtype=np.float32)
        half_mem = (1.0 - RATE) * mem
        for t in range(T):
            lo = RT * t - HALF
            a, bnd = max(0, lo), min(N, lo + 128)
            if a < bnd:
                mnb[t, a - lo:bnd - lo, 0:L] = half_mem[a:bnd]
                mnb[t, a - lo:bnd - lo, L] = 1.0
        xh = np.zeros((T, RT, L), dtype=np.float32)
        xh.reshape(-1, L)[:N] = RATE * x
        in_maps.append({
            "xmA": np.ascontiguousarray(np.concatenate(
                [wT, x.T[:, 0:1024], mem.T[:, 0:1024]], axis=1)),
            "xmB": np.ascontiguousarray(np.concatenate(
                [x.T[:, 1024:2048], mem.T[:, 1024:2048]], axis=1)),
            "idt": np.eye(RT, dtype=np.float32).astype(bfloat16),
            "scb": scb,
            "maskG": maskG.astype(bfloat16),
            "memNB": np.ascontiguousarray(
                mnb.transpose(1, 0, 2).reshape(128, -1)).astype(bfloat16),
            "xhp": np.ascontiguousarray(xh.transpose(1, 0, 2).reshape(RT, -1)),
        })
    return in_maps


def run(inputs, trace=False):
    from concourse.bass_utils import run_bass_kernel_spmd
    if "nc" not in _cache:
        _cache["nc"] = _build_program()
    nc = _cache["nc"]
    in_maps = _host_prep(**inputs)
    res = run_bass_kernel_spmd(nc, in_maps, core_ids=list(range(B)), trace=trace)
    out = np.empty((B, N, C, L), dtype=np.float32)
    for b in range(B):
        o = res.results[b]["out"].reshape(RT, T, L).transpose(1, 0, 2)
        out[b] = o.reshape(NPAD, L)[:N].reshape(N, C, L)
    return out, res


def kernel(**inputs):
    out, _ = run(inputs, trace=False)
    return out
